# revision 18
# baseline (speedup 1.0000x reference)
"""MoE gate (top-6 routing) Trainium2 Bass kernel, v6.

Problem: hidden_states [4, 4096, 2048] f32, gate weight [64, 2048] f32.
  logits = x @ W.T            -> [16384, 64]
  topk_weight, topk_idx = top_k(logits, 6)
  topk_weight = softmax(topk_weight)
Returns (topk_idx int32 [16384, 6], topk_weight f32 [16384, 6]).

Sharding: data-parallel over tokens; 2048 tokens/core, weight replicated.

Precision (identical math to the verified baseline): x and w are split on
the host into fp16 halves, v = vh + 2^-11*vl, giving ~2^-22 relative
precision. logits = xh@wh + 2^-11*(xh@wl + xl@wh), bit-level top-6
agreement with the fp32 reference on the test inputs.

Structure (evidence-driven over five traced iterations):
  - Column-group concurrency: each token segment is split into two
    halves assigned to PE column groups 0/1 (psum partitions 0:64 /
    64:128). Two matmuls in different column groups stream concurrently
    (~54 ns/matmul effective at N=256 vs 379 ns full-width), so the PE
    easily tracks the DMA stream. Stationaries are the 64-wide wh / wl
    k-tiles; per (segment, k-tile): xh@wh -> psM, xh@wl -> psC,
    xl@wh -> psC on both halves.
  - Segment-major streaming on a single HWDGE queue (a second SWDGE
    queue measured SLOWER in aggregate): ~1 MiB chunks; the per-token
    segments are 512,512,512,256,256 so the final epilogue (which can't
    overlap any stream) only covers 256 tokens.
  - HAM warmup: 16 junk matmuls (~7 us cold) cover a full clock-gate
    activity window at any phase; the PE then runs at 2.4 GHz when the
    first data lands.
  - Per-segment epilogue overlaps the next segment's stream: ACT
    copy-scale + DVE add combine (one PSUM input per op), PE-transpose
    to [token, expert], DVE max8/max_index, ACT exp(accum_out), DVE
    reciprocal, normalize alternating ACT/DVE. One output DMA per
    segment from a single u32 stage tile.
"""

import numpy as np

import concourse.mybir as mybir
import concourse.tile as tile
from concourse import bacc
from concourse.bass_utils import run_bass_kernel_spmd

f32 = mybir.dt.float32
f16 = mybir.dt.float16
u32 = mybir.dt.uint32
i32 = mybir.dt.int32

N_CORES = 8
B, S, H = 4, 4096, 2048
E = 64
TOP_K = 6
T_FULL = B * S              # 16384 tokens
T_CORE = T_FULL // N_CORES  # 2048 tokens per core
KT = H // 128               # 16 contraction tiles
NTT = T_CORE // 128         # 16 token tiles per core
OC = TOP_K + 8              # 14 staged u32 cols per token tile (6 w + 8 idx)
LSCALE = float(2.0 ** -11)
# token segments (start, size): the tail segments are small so the final
# top-k chain after the last byte covers only 256 tokens
SEGS = [(0, 512), (512, 512), (1024, 512), (1536, 256), (1792, 256)]
# k-tiles per DMA chunk per segment (one k-tile of a segment = size/2
# fp16 cols in each half => size*4 bytes... chunk bytes = ksz*size*4)
SEG_CHUNKS = [[2, 2, 2, 2, 4, 4], [4, 4, 4, 4], [4, 4, 4, 4], [4, 4, 4, 4], [4, 4, 4, 4]]

_CACHE = {}


def _seg_base(si):
    # column offset of segment si in the stream tensor
    return sum(KT * 2 * sz for _, sz in SEGS[:si])


XCOLS = sum(KT * 2 * sz for _, sz in SEGS)  # 65536


def _build():
    nc = bacc.Bacc("TRN2", target_bir_lowering=False, debug=False)
    # x stream [128, 65536] fp16: segment si, k-tile a occupies cols
    # base(si) + a*2*size + [0:size xh | size:2*size xl]
    x = nc.dram_tensor("x", [128, XCOLS], f16, kind="ExternalInput").ap()
    # weight pack [128, KT*128]: cols a*128+j = wh[j] (j<64) | wl[j-64]
    w2 = nc.dram_tensor("w2", [128, KT * 128], f16, kind="ExternalInput").ap()
    ident = nc.dram_tensor("ident", [E, E], f32, kind="ExternalInput").ap()
    out = nc.dram_tensor("out", [128, NTT * OC], u32, kind="ExternalOutput").ap()

    with tile.TileContext(nc) as tc:
        with (
            tc.tile_pool(name="persist", bufs=1) as persist,
            tc.tile_pool(name="work", bufs=4) as work,
            tc.tile_pool(name="psM", bufs=2, space="PSUM") as psMp,
            tc.tile_pool(name="psC", bufs=2, space="PSUM") as psCp,
            tc.tile_pool(name="psT", bufs=2, space="PSUM") as psTp,
            tc.tile_pool(name="psW", bufs=1, space="PSUM") as psWp,
        ):
            # ---- input DMAs in stream order; first k-tiles' weights and
            #      first chunks lead so matmuls start as the ramp allows
            w2_t = persist.tile([128, KT * 128], f16, tag="w2")
            id_t = persist.tile([E, E], f32, tag="ident")
            xat = {}  # (si, a) -> (tile, col offset)
            xtiles = []
            for si, (t0, sz) in enumerate(SEGS):
                a0 = 0
                base = _seg_base(si)
                for ci, ksz in enumerate(SEG_CHUNKS[si]):
                    t = persist.tile([128, ksz * 2 * sz], f16, tag=f"x{si}_{ci}")
                    src0 = base + a0 * 2 * sz
                    xtiles.append((t, src0, ksz * 2 * sz))
                    for j in range(ksz):
                        xat[(si, a0 + j)] = (t, j * 2 * sz)
                    a0 += ksz
            nc.sync.dma_start(out=w2_t[:, 0:256], in_=w2[:, 0:256])
            for t, src0, ncols in xtiles[:2]:
                nc.sync.dma_start(out=t, in_=x[:, src0 : src0 + ncols])
            nc.sync.dma_start(out=w2_t[:, 256:], in_=w2[:, 256:])
            nc.sync.dma_start(out=id_t, in_=ident)
            for t, src0, ncols in xtiles[2:]:
                nc.sync.dma_start(out=t, in_=x[:, src0 : src0 + ncols])

            stage = persist.tile([128, NTT * OC], u32, tag="stage")

            # ---- HAM warmup: ~7 us of junk matmuls covers a full activity
            #      window at any phase, opening the PE clock gate ----
            wm = persist.tile([128, 512], f16, tag="warm")
            nc.gpsimd.memset(wm, 0.25)
            wps = psWp.tile([128, 512], f32, tag="wps")
            for _ in range(22):
                nc.tensor.matmul(wps, wm[:, 0:128], wm, start=True, stop=True)

            def emit_k(psM, psC, si, a):
                sz = SEGS[si][1]
                hb = sz // 2
                ch, o = xat[(si, a)]
                wh = w2_t[:, a * 128 : a * 128 + 64]
                wl = w2_t[:, a * 128 + 64 : (a + 1) * 128]
                first, last = a == 0, a == KT - 1
                for g in range(2):
                    pr = slice(g * 64, (g + 1) * 64)
                    xh = ch[:, o + g * hb : o + (g + 1) * hb]
                    nc.tensor.matmul(psM[pr, :], wh, xh, start=first, stop=last)
                for g in range(2):
                    pr = slice(g * 64, (g + 1) * 64)
                    xh = ch[:, o + g * hb : o + (g + 1) * hb]
                    nc.tensor.matmul(psC[pr, :], wl, xh, start=first, stop=False)
                for g in range(2):
                    pr = slice(g * 64, (g + 1) * 64)
                    xl = ch[:, o + sz + g * hb : o + sz + (g + 1) * hb]
                    nc.tensor.matmul(psC[pr, :], wh, xl, start=False, stop=last)

            def emit_combine(psM, psC, si):
                # lt[:, g*hb:+hb] = psM[g] + 2^-11 * psC[g] per half.
                # (An op may read at most one PSUM input, so stage the
                # scaled correction through SBUF on the Scalar engine.)
                sz = SEGS[si][1]
                hb = sz // 2
                lt = work.tile([E, sz], f32, tag="lt")
                for g in range(2):
                    pr = slice(g * 64, (g + 1) * 64)
                    s = work.tile([E, hb], f32, tag="s")
                    nc.scalar.activation(
                        out=s, in_=psC[pr, :],
                        func=mybir.ActivationFunctionType.Copy, scale=LSCALE,
                    )
                    nc.vector.tensor_add(
                        lt[:, g * hb : (g + 1) * hb], s, psM[pr, :]
                    )
                return lt

            def emit_topk(si, lt):
                t0, sz = SEGS[si]
                for tt in range(sz // 128):
                    t = t0 // 128 + tt
                    ps_t = psTp.tile([128, E], f32, tag="ps_t")
                    nc.tensor.transpose(ps_t, lt[:, tt * 128 : (tt + 1) * 128], id_t)
                    m8 = work.tile([128, 8], f32, tag="m8")
                    nc.vector.max(out=m8, in_=ps_t)
                    nc.vector.max_index(
                        stage[:, t * OC + TOP_K : (t + 1) * OC], m8, ps_t
                    )
                    expw = work.tile([128, TOP_K], f32, tag="expw")
                    ssum = work.tile([128, 1], f32, tag="ssum")
                    nc.scalar.activation(
                        out=expw,
                        in_=m8[:, 0:TOP_K],
                        func=mybir.ActivationFunctionType.Exp,
                        accum_out=ssum[:, 0:1],
                    )
                    rsum = work.tile([128, 1], f32, tag="rsum")
                    nc.vector.reciprocal(rsum, ssum)
                    # alternate the normalize between ACT and DVE so
                    # neither engine serializes the per-tile chains
                    if tt % 2 == 0:
                        nc.scalar.activation(
                            out=stage[:, t * OC : t * OC + TOP_K].bitcast(f32),
                            in_=expw,
                            func=mybir.ActivationFunctionType.Copy,
                            scale=rsum[:, 0:1],
                        )
                    else:
                        nc.vector.tensor_scalar_mul(
                            stage[:, t * OC : t * OC + TOP_K].bitcast(f32),
                            expw,
                            rsum[:, 0:1],
                        )
                # outputs ride the ACT HWDGE ring: their completion
                # stalls must never interrupt the x stream on the Sync
                # ring (measured: they caused 3 us end-of-stream gaps)
                c0 = (t0 // 128) * OC
                c1 = ((t0 + sz) // 128) * OC
                nc.scalar.dma_start(out=out[:, c0:c1], in_=stage[:, c0:c1])

            pending = None  # (si, lt) whose transposes/top-k are deferred
            for si in range(len(SEGS)):
                hb = SEGS[si][1] // 2
                psM = psMp.tile([128, hb], f32, tag="psM")
                psC = psCp.tile([128, hb], f32, tag="psC")
                for a in range(KT):
                    emit_k(psM, psC, si, a)
                    # previous segment's PE transposes + top-k go a few
                    # k-tiles into this segment so their ACT/DVE combine
                    # inputs are ready and the PE never stalls on them.
                    if a == 5 and pending is not None:
                        emit_topk(*pending)
                        pending = None
                pending = (si, emit_combine(psM, psC, si))
            emit_topk(*pending)

    nc.compile()
    return nc


def _get_nc():
    if "nc" not in _CACHE:
        _CACHE["nc"] = _build()
    return _CACHE["nc"]


def _split_fp16(arr32):
    """arr32 (fp32) -> (hi fp16, lo fp16) with arr32 ~= hi + 2^-11 * lo."""
    hi = arr32.astype(np.float16)
    lo = ((arr32 - hi.astype(np.float32)) * 2048.0).astype(np.float16)
    return hi, lo


def kernel(hidden_states: np.ndarray, weight: np.ndarray, **_run_kwargs):
    x = np.ascontiguousarray(hidden_states, dtype=np.float32).reshape(T_FULL, H)
    w = np.ascontiguousarray(weight, dtype=np.float32)

    w_hi, w_lo = _split_fp16(w)  # [E, H] fp16
    # [128, KT*128]: row p, col a*128+j  <-  (wh|wl)[j, a*128+p]
    wh_r = np.ascontiguousarray(w_hi.T).reshape(KT, 128, E)
    wl_r = np.ascontiguousarray(w_lo.T).reshape(KT, 128, E)
    w2p = np.ascontiguousarray(
        np.concatenate([wh_r, wl_r], axis=2).transpose(1, 0, 2).reshape(128, KT * 128)
    )
    ident = np.eye(E, dtype=np.float32)

    def pack_x(xT16_h, xT16_l):
        # [H, T_CORE] halves -> [128, XCOLS] segment stream order
        vh = xT16_h.reshape(KT, 128, T_CORE)  # [a, p, t]
        vl = xT16_l.reshape(KT, 128, T_CORE)
        X = np.empty((128, XCOLS), dtype=np.float16)
        for si, (t0, sz) in enumerate(SEGS):
            base = _seg_base(si)
            for a in range(KT):
                c = base + a * 2 * sz
                X[:, c : c + sz] = vh[a, :, t0 : t0 + sz]
                X[:, c + sz : c + 2 * sz] = vl[a, :, t0 : t0 + sz]
        return X

    in_maps = []
    for c in range(N_CORES):
        shard = x[c * T_CORE : (c + 1) * T_CORE, :]  # [T_CORE, H]
        xT = np.ascontiguousarray(shard.T)  # [H, T_CORE] fp32
        xhs, xls = _split_fp16(xT)
        in_maps.append({"x": pack_x(xhs, xls), "w2": w2p, "ident": ident})

    nc = _get_nc()
    res = run_bass_kernel_spmd(
        nc, in_maps, core_ids=list(range(N_CORES)), **_run_kwargs
    )

    idx_parts = []
    w_parts = []
    for c in range(N_CORES):
        r = res.results[c]
        v = r["out"].reshape(128, NTT, OC).transpose(1, 0, 2)  # [tile, tok, col]
        idx = np.ascontiguousarray(v[:, :, TOP_K : TOP_K + TOP_K])
        wts = np.ascontiguousarray(v[:, :, 0:TOP_K]).view(np.uint32)
        idx_parts.append(
            idx.reshape(T_CORE, TOP_K).astype(np.int32, copy=False)
        )
        w_parts.append(wts.view(np.float32).reshape(T_CORE, TOP_K))

    topk_idx = np.concatenate(idx_parts, axis=0)
    topk_weight = np.concatenate(w_parts, axis=0)
    if "trace" in _run_kwargs:
        return (topk_idx, topk_weight), res
    return topk_idx, topk_weight


# revision 22
# speedup vs baseline: 1.0456x; 1.0456x over previous
"""MoE gate (top-6 routing) Trainium2 Bass kernel, v6.

Problem: hidden_states [4, 4096, 2048] f32, gate weight [64, 2048] f32.
  logits = x @ W.T            -> [16384, 64]
  topk_weight, topk_idx = top_k(logits, 6)
  topk_weight = softmax(topk_weight)
Returns (topk_idx int32 [16384, 6], topk_weight f32 [16384, 6]).

Sharding: data-parallel over tokens; 2048 tokens/core, weight replicated.

Precision (identical math to the verified baseline): x and w are split on
the host into fp16 halves, v = vh + 2^-11*vl, giving ~2^-22 relative
precision. logits = xh@wh + 2^-11*(xh@wl + xl@wh), bit-level top-6
agreement with the fp32 reference on the test inputs.

Structure (evidence-driven over five traced iterations):
  - Column-group concurrency: each token segment is split into two
    halves assigned to PE column groups 0/1 (psum partitions 0:64 /
    64:128). Two matmuls in different column groups stream concurrently
    (~54 ns/matmul effective at N=256 vs 379 ns full-width), so the PE
    easily tracks the DMA stream. Stationaries are the 64-wide wh / wl
    k-tiles; per (segment, k-tile): xh@wh -> psM, xh@wl -> psC,
    xl@wh -> psC on both halves.
  - Segment-major streaming on a single HWDGE queue (a second SWDGE
    queue measured SLOWER in aggregate): ~1 MiB chunks; the per-token
    segments are 512,512,512,256,256 so the final epilogue (which can't
    overlap any stream) only covers 256 tokens.
  - HAM warmup: 16 junk matmuls (~7 us cold) cover a full clock-gate
    activity window at any phase; the PE then runs at 2.4 GHz when the
    first data lands.
  - Per-segment epilogue overlaps the next segment's stream: ACT
    copy-scale + DVE add combine (one PSUM input per op), PE-transpose
    to [token, expert], DVE max8/max_index, ACT exp(accum_out), DVE
    reciprocal, normalize alternating ACT/DVE. One output DMA per
    segment from a single u32 stage tile.
"""

import numpy as np

import concourse.mybir as mybir
import concourse.tile as tile
from concourse import bacc
from concourse.bass_utils import run_bass_kernel_spmd

f32 = mybir.dt.float32
f16 = mybir.dt.float16
u32 = mybir.dt.uint32
i32 = mybir.dt.int32

N_CORES = 8
B, S, H = 4, 4096, 2048
E = 64
TOP_K = 6
T_FULL = B * S              # 16384 tokens
T_CORE = T_FULL // N_CORES  # 2048 tokens per core
KT = H // 128               # 16 contraction tiles
NTT = T_CORE // 128         # 16 token tiles per core
OC = TOP_K + 8              # 14 staged u32 cols per token tile (6 w + 8 idx)
LSCALE = float(2.0 ** -11)
# token segments (start, size): the tail segments are small so the final
# top-k chain after the last byte covers only 256 tokens
SEGS = [(0, 512), (512, 512), (1024, 512), (1536, 256), (1792, 256)]
# k-tiles per DMA chunk per segment (one k-tile of a segment = size/2
# fp16 cols in each half => size*4 bytes... chunk bytes = ksz*size*4)
SEG_CHUNKS = [[2, 2, 2, 2, 4, 4], [4, 4, 4, 4], [4, 4, 4, 4], [8, 8], [8, 8]]

_CACHE = {}


def _seg_base(si):
    # column offset of segment si in the stream tensor
    return sum(KT * 2 * sz for _, sz in SEGS[:si])


XCOLS = sum(KT * 2 * sz for _, sz in SEGS)  # 65536


def _build():
    nc = bacc.Bacc("TRN2", target_bir_lowering=False, debug=False)
    # x stream [128, 65536] fp16: segment si, k-tile a occupies cols
    # base(si) + a*2*size + [0:size xh | size:2*size xl]
    x = nc.dram_tensor("x", [128, XCOLS], f16, kind="ExternalInput").ap()
    # weight pack [128, KT*128]: cols a*128+j = wh[j] (j<64) | wl[j-64]
    w2 = nc.dram_tensor("w2", [128, KT * 128], f16, kind="ExternalInput").ap()
    ident = nc.dram_tensor("ident", [E, E], f32, kind="ExternalInput").ap()
    out = nc.dram_tensor("out", [128, NTT * OC], u32, kind="ExternalOutput").ap()

    with tile.TileContext(nc) as tc:
        with (
            tc.tile_pool(name="persist", bufs=1) as persist,
            tc.tile_pool(name="work", bufs=4) as work,
            tc.tile_pool(name="psM", bufs=2, space="PSUM") as psMp,
            tc.tile_pool(name="psC", bufs=2, space="PSUM") as psCp,
            tc.tile_pool(name="psT", bufs=2, space="PSUM") as psTp,
            tc.tile_pool(name="psW", bufs=1, space="PSUM") as psWp,
        ):
            # ---- input DMAs in stream order; first k-tiles' weights and
            #      first chunks lead so matmuls start as the ramp allows
            w2_t = persist.tile([128, KT * 128], f16, tag="w2")
            id_t = persist.tile([E, E], f32, tag="ident")
            xat = {}  # (si, a) -> (tile, col offset)
            xtiles = []
            for si, (t0, sz) in enumerate(SEGS):
                a0 = 0
                base = _seg_base(si)
                for ci, ksz in enumerate(SEG_CHUNKS[si]):
                    t = persist.tile([128, ksz * 2 * sz], f16, tag=f"x{si}_{ci}")
                    src0 = base + a0 * 2 * sz
                    xtiles.append((t, src0, ksz * 2 * sz))
                    for j in range(ksz):
                        xat[(si, a0 + j)] = (t, j * 2 * sz)
                    a0 += ksz
            nc.sync.dma_start(out=w2_t[:, 0:256], in_=w2[:, 0:256])
            for t, src0, ncols in xtiles[:2]:
                nc.sync.dma_start(out=t, in_=x[:, src0 : src0 + ncols])
            nc.sync.dma_start(out=w2_t[:, 256:], in_=w2[:, 256:])
            nc.sync.dma_start(out=id_t, in_=ident)
            for t, src0, ncols in xtiles[2:]:
                nc.sync.dma_start(out=t, in_=x[:, src0 : src0 + ncols])

            stage = persist.tile([128, NTT * OC], u32, tag="stage")

            # ---- HAM warmup: ~7 us of junk matmuls covers a full activity
            #      window at any phase, opening the PE clock gate ----
            wm = persist.tile([128, 512], f16, tag="warm")
            nc.gpsimd.memset(wm, 0.25)
            wps = psWp.tile([128, 512], f32, tag="wps")
            for _ in range(16):
                nc.tensor.matmul(wps, wm[:, 0:128], wm, start=True, stop=True)

            def emit_k_full(psA, psB, a):
                # Segment 0 runs FULL-WIDTH [wh|wl] stationaries (slower
                # 379 ns/MM): during the DMA ramp the PE is data-starved,
                # so denser-but-slower PE work keeps its duty near 100%
                # and the HAM clock gate provably open, at no cost.
                ch, o = xat[(0, a)]
                lhs = w2_t[:, a * 128 : (a + 1) * 128]
                first, last = a == 0, a == KT - 1
                nc.tensor.matmul(
                    psA, lhs, ch[:, o : o + 512], start=first, stop=last
                )
                nc.tensor.matmul(
                    psB, lhs, ch[:, o + 512 : o + 1024], start=first, stop=last
                )

            def emit_combine_full(psA, psB):
                # lt = psA[0:64] + 2^-11*(psA[64:128] + psB[0:64])
                # (psA[64:128]=xh@wl, psB[0:64]=xl@wh; psB[64:128]=xl@wl
                # is the dropped 4th-order term)
                lt = work.tile([E, 512], f32, tag="lt")
                s = work.tile([E, 512], f32, tag="s")
                nc.scalar.activation(
                    out=s, in_=psA[E:128, :],
                    func=mybir.ActivationFunctionType.Copy, scale=LSCALE,
                )
                t = work.tile([E, 512], f32, tag="t2")
                nc.vector.scalar_tensor_tensor(
                    t, psB[0:E, :], LSCALE, s,
                    mybir.AluOpType.mult, mybir.AluOpType.add,
                )
                nc.vector.tensor_add(lt, t, psA[0:E, :])
                return lt

            def emit_k(psM, psC, si, a):
                sz = SEGS[si][1]
                hb = sz // 2
                ch, o = xat[(si, a)]
                wh = w2_t[:, a * 128 : a * 128 + 64]
                wl = w2_t[:, a * 128 + 64 : (a + 1) * 128]
                first, last = a == 0, a == KT - 1
                for g in range(2):
                    pr = slice(g * 64, (g + 1) * 64)
                    xh = ch[:, o + g * hb : o + (g + 1) * hb]
                    nc.tensor.matmul(psM[pr, :], wh, xh, start=first, stop=last)
                for g in range(2):
                    pr = slice(g * 64, (g + 1) * 64)
                    xh = ch[:, o + g * hb : o + (g + 1) * hb]
                    nc.tensor.matmul(psC[pr, :], wl, xh, start=first, stop=False)
                for g in range(2):
                    pr = slice(g * 64, (g + 1) * 64)
                    xl = ch[:, o + sz + g * hb : o + sz + (g + 1) * hb]
                    nc.tensor.matmul(psC[pr, :], wh, xl, start=False, stop=last)

            def emit_combine(psM, psC, si):
                # lt[:, g*hb:+hb] = psM[g] + 2^-11 * psC[g] per half.
                # (An op may read at most one PSUM input, so stage the
                # scaled correction through SBUF on the Scalar engine.)
                sz = SEGS[si][1]
                hb = sz // 2
                lt = work.tile([E, sz], f32, tag="lt")
                for g in range(2):
                    pr = slice(g * 64, (g + 1) * 64)
                    s = work.tile([E, hb], f32, tag="s")
                    nc.scalar.activation(
                        out=s, in_=psC[pr, :],
                        func=mybir.ActivationFunctionType.Copy, scale=LSCALE,
                    )
                    nc.vector.tensor_add(
                        lt[:, g * hb : (g + 1) * hb], s, psM[pr, :]
                    )
                return lt

            def emit_topk(si, lt):
                t0, sz = SEGS[si]
                for tt in range(sz // 128):
                    t = t0 // 128 + tt
                    ps_t = psTp.tile([128, E], f32, tag="ps_t")
                    nc.tensor.transpose(ps_t, lt[:, tt * 128 : (tt + 1) * 128], id_t)
                    m8 = work.tile([128, 8], f32, tag="m8")
                    nc.vector.max(out=m8, in_=ps_t)
                    nc.vector.max_index(
                        stage[:, t * OC + TOP_K : (t + 1) * OC], m8, ps_t
                    )
                    expw = work.tile([128, TOP_K], f32, tag="expw")
                    ssum = work.tile([128, 1], f32, tag="ssum")
                    nc.scalar.activation(
                        out=expw,
                        in_=m8[:, 0:TOP_K],
                        func=mybir.ActivationFunctionType.Exp,
                        accum_out=ssum[:, 0:1],
                    )
                    rsum = work.tile([128, 1], f32, tag="rsum")
                    nc.vector.reciprocal(rsum, ssum)
                    # alternate the normalize between ACT and DVE so
                    # neither engine serializes the per-tile chains
                    if tt % 2 == 0:
                        nc.scalar.activation(
                            out=stage[:, t * OC : t * OC + TOP_K].bitcast(f32),
                            in_=expw,
                            func=mybir.ActivationFunctionType.Copy,
                            scale=rsum[:, 0:1],
                        )
                    else:
                        nc.vector.tensor_scalar_mul(
                            stage[:, t * OC : t * OC + TOP_K].bitcast(f32),
                            expw,
                            rsum[:, 0:1],
                        )
                # outputs ride the ACT HWDGE ring: their completion
                # stalls must never interrupt the x stream on the Sync
                # ring (measured: they caused 3 us end-of-stream gaps)
                c0 = (t0 // 128) * OC
                c1 = ((t0 + sz) // 128) * OC
                nc.scalar.dma_start(out=out[:, c0:c1], in_=stage[:, c0:c1])

            pending = None  # (si, lt) whose transposes/top-k are deferred
            for si in range(len(SEGS)):
                if si == 0:
                    psA = psMp.tile([128, 512], f32, tag="psM")
                    psB = psCp.tile([128, 512], f32, tag="psC")
                    for a in range(KT):
                        emit_k_full(psA, psB, a)
                    pending = (0, emit_combine_full(psA, psB))
                    continue
                hb = SEGS[si][1] // 2
                psM = psMp.tile([128, hb], f32, tag="psM")
                psC = psCp.tile([128, hb], f32, tag="psC")
                for a in range(KT):
                    emit_k(psM, psC, si, a)
                    # previous segment's PE transposes + top-k go a few
                    # k-tiles into this segment so their ACT/DVE combine
                    # inputs are ready and the PE never stalls on them.
                    if a == 5 and pending is not None:
                        emit_topk(*pending)
                        pending = None
                pending = (si, emit_combine(psM, psC, si))
            emit_topk(*pending)

    nc.compile()
    return nc


def _get_nc():
    if "nc" not in _CACHE:
        _CACHE["nc"] = _build()
    return _CACHE["nc"]


def _split_fp16(arr32):
    """arr32 (fp32) -> (hi fp16, lo fp16) with arr32 ~= hi + 2^-11 * lo."""
    hi = arr32.astype(np.float16)
    lo = ((arr32 - hi.astype(np.float32)) * 2048.0).astype(np.float16)
    return hi, lo


def kernel(hidden_states: np.ndarray, weight: np.ndarray, **_run_kwargs):
    x = np.ascontiguousarray(hidden_states, dtype=np.float32).reshape(T_FULL, H)
    w = np.ascontiguousarray(weight, dtype=np.float32)

    w_hi, w_lo = _split_fp16(w)  # [E, H] fp16
    # [128, KT*128]: row p, col a*128+j  <-  (wh|wl)[j, a*128+p]
    wh_r = np.ascontiguousarray(w_hi.T).reshape(KT, 128, E)
    wl_r = np.ascontiguousarray(w_lo.T).reshape(KT, 128, E)
    w2p = np.ascontiguousarray(
        np.concatenate([wh_r, wl_r], axis=2).transpose(1, 0, 2).reshape(128, KT * 128)
    )
    ident = np.eye(E, dtype=np.float32)

    def pack_x(xT16_h, xT16_l):
        # [H, T_CORE] halves -> [128, XCOLS] segment stream order
        vh = xT16_h.reshape(KT, 128, T_CORE)  # [a, p, t]
        vl = xT16_l.reshape(KT, 128, T_CORE)
        X = np.empty((128, XCOLS), dtype=np.float16)
        for si, (t0, sz) in enumerate(SEGS):
            base = _seg_base(si)
            for a in range(KT):
                c = base + a * 2 * sz
                X[:, c : c + sz] = vh[a, :, t0 : t0 + sz]
                X[:, c + sz : c + 2 * sz] = vl[a, :, t0 : t0 + sz]
        return X

    in_maps = []
    for c in range(N_CORES):
        shard = x[c * T_CORE : (c + 1) * T_CORE, :]  # [T_CORE, H]
        xT = np.ascontiguousarray(shard.T)  # [H, T_CORE] fp32
        xhs, xls = _split_fp16(xT)
        in_maps.append({"x": pack_x(xhs, xls), "w2": w2p, "ident": ident})

    nc = _get_nc()
    res = run_bass_kernel_spmd(
        nc, in_maps, core_ids=list(range(N_CORES)), **_run_kwargs
    )

    idx_parts = []
    w_parts = []
    for c in range(N_CORES):
        r = res.results[c]
        v = r["out"].reshape(128, NTT, OC).transpose(1, 0, 2)  # [tile, tok, col]
        idx = np.ascontiguousarray(v[:, :, TOP_K : TOP_K + TOP_K])
        wts = np.ascontiguousarray(v[:, :, 0:TOP_K]).view(np.uint32)
        idx_parts.append(
            idx.reshape(T_CORE, TOP_K).astype(np.int32, copy=False)
        )
        w_parts.append(wts.view(np.float32).reshape(T_CORE, TOP_K))

    topk_idx = np.concatenate(idx_parts, axis=0)
    topk_weight = np.concatenate(w_parts, axis=0)
    if "trace" in _run_kwargs:
        return (topk_idx, topk_weight), res
    return topk_idx, topk_weight
